# revision 8
# baseline (speedup 1.0000x reference)
"""DiscretizedMixLogisticLoss Bass kernel for TRN2, 8-core data-parallel.

Full inputs: x [8,3,256,256] f32, l [8,120,256,256] f32 -> nll [8,3,256,256] f32.
Sharding: batch dim N=8 across 8 cores (1 example/core).

Math (per pixel, channel c, mixture k), with l viewed as [4,3,10,H*W]:
  s = l[0], mu = l[1], sc = l[2], co = l[3]
  sig3 = sigmoid(co); m' = clip(mu + coupling(sig3*x), 0, 255)
  inv = exp(-sc)
  t_lo = (x-0.5-m')*inv ; t_hi = t_lo + inv
  d = max(sigmoid(t_hi)-sigmoid(t_lo), 1e-12)
  nll = log(sum_k e^s) - log(sum_k e^s * d)
sigmoid(t) = 1/(1+exp(-t)) with the division via reciprocal_approx_accurate
(~2 ULP); the inf-guard v = min(u,1e30)+1 is fused into one tensor_scalar.
Edge pixels (x<0.001 / x>254.999, ~13 of 1.5M) are fixed up on host.

Layout per core: partition p = (c*10+k)*4 + q, where q indexes the four
16384-pixel quarters; free dim = J pixels per tile. K-sums ride TensorE via a
0/1 selection matmul (bf16 operands) into PSUM.
"""
from contextlib import ExitStack

import numpy as np

import concourse.bass as bass
import concourse.bacc as bacc
import concourse.tile as tile
from concourse import mybir
from concourse.bass_utils import run_bass_kernel_spmd

AF = mybir.ActivationFunctionType
ALU = mybir.AluOpType
F32 = mybir.dt.float32
BF16 = mybir.dt.bfloat16

N, C, K, H, W = 8, 3, 10, 256, 256
HW = H * W            # 65536 pixels per example
CK = C * K            # 30
P = CK * 4            # 120 partitions used
NCORES = 8

J = 1024              # pixels per partition per tile
T = 4 * J             # pixels per tile
NT = HW // T          # tiles per core (16)
QS = HW // 4          # 16384 pixel-quarter stride


def build_kernel():
    nc = bacc.Bacc("TRN2", target_bir_lowering=False, debug=False)

    l_in = nc.dram_tensor("l_in", [4 * CK, HW], F32, kind="ExternalInput")
    xhl_in = nc.dram_tensor("xhl_in", [2 * C, HW], BF16, kind="ExternalInput")
    selsum_d = nc.dram_tensor("selsum_c", [P, 120], F32, kind="ExternalInput")
    selb_d = nc.dram_tensor("selb_c", [P, 264], BF16, kind="ExternalInput")
    out = nc.dram_tensor("out", [C, HW], F32, kind="ExternalOutput")

    with tile.TileContext(nc) as tc, ExitStack() as ctx:
        consts = ctx.enter_context(tc.tile_pool(name="consts", bufs=1))
        lpool = ctx.enter_context(tc.tile_pool(name="lpool", bufs=3))
        xpool = ctx.enter_context(tc.tile_pool(name="xpool", bufs=3))
        wpool = ctx.enter_context(tc.tile_pool(name="wpool", bufs=3))
        opool = ctx.enter_context(tc.tile_pool(name="opool", bufs=3))
        psum = ctx.enter_context(tc.tile_pool(name="psum", bufs=2, space="PSUM"))
        stand = ctx.enter_context(tc.tile_pool(name="stand", bufs=1))

        selsum = consts.tile([P, 120], F32)
        nc.sync.dma_start(out=selsum, in_=selsum_d[:, :])
        selb = consts.tile([P, 264], BF16)
        nc.sync.dma_start(out=selb, in_=selb_d[:, :])
        w_coup = selsum[:, 0:120]
        w_r1 = selb[:, 0:12]
        w_r2 = selb[:, 12:24]
        w_b1 = selb[0:12, 24:144]
        w_b2 = selb[0:12, 144:264]

        # standing results, two groups so the ln-tail of group A overlaps
        # group B's main loop. partition = (t - t0)*12 + (c*4+q).
        NTA = 8
        rbufA = stand.tile([NTA * 12, 2 * J], F32)
        rbufB = stand.tile([(NT - NTA) * 12, 2 * J], F32)

        def _tail(rb, t0, nt):
            nc.scalar.activation(out=rb, in_=rb, func=AF.Ln)
            nc.vector.tensor_tensor(out=rb[:, 0:J], in0=rb[:, 0:J],
                                    in1=rb[:, J:2 * J], op=ALU.subtract)
            for tt_ in range(nt):
                nc.sync.dma_start(
                    out=bass.AP(tensor=out, offset=(t0 + tt_) * J,
                                ap=[[HW, C], [QS, 4], [1, J]]),
                    in_=rb[tt_ * 12:(tt_ + 1) * 12, 0:J])

        for t in range(NT):
            # merged 4-param l DMA: lt[:, p*J:(p+1)*J] = param p at tile t
            lt = lpool.tile([P, 4 * J], F32, tag="lt")
            nc.sync.dma_start(
                out=lt,
                in_=bass.AP(tensor=l_in, offset=t * J,
                            ap=[[HW, CK], [QS, 4], [CK * HW, 4], [1, J]]))
            lg = lt[:, 0:J]
            mu = lt[:, J:2 * J]
            sc = lt[:, 2 * J:3 * J]
            co = lt[:, 3 * J:4 * J]

            # x quarters [12, 2J] bf16: [0:J]=hi, [J:2J]=lo (partition = c*4+q)
            xq = xpool.tile([12, 2 * J], BF16, tag="xq")
            nc.sync.dma_start(
                out=xq,
                in_=bass.AP(tensor=xhl_in, offset=t * J,
                            ap=[[HW, C], [QS, 4], [C * HW, 2], [1, J]]))
            cxb = psum.tile([P, J], F32, tag="pa")  # coupling x: (x0,x0,x1)
            for i in range(J // 512):
                s0, s1 = i * 512, (i + 1) * 512
                nc.tensor.matmul(cxb[:, s0:s1], w_b2, xq[:, s0:s1],
                                 start=True, stop=False)
                nc.tensor.matmul(cxb[:, s0:s1], w_b2, xq[:, J + s0:J + s1],
                                 start=False, stop=True)

            # xb bcast via PE (early: only needs xq), hi+lo accumulate
            xb = psum.tile([P, J], F32, tag="pb")
            for i in range(J // 512):
                s0, s1 = i * 512, (i + 1) * 512
                nc.tensor.matmul(xb[:, s0:s1], w_b1, xq[:, s0:s1],
                                 start=True, stop=False)
                nc.tensor.matmul(xb[:, s0:s1], w_b1, xq[:, J + s0:J + s1],
                                 start=False, stop=True)

            # sig3 = 0.5*tanh(0.5*co)+0.5 ; th3 = sig3*cxb (in place on co)
            nc.scalar.activation(out=co, in_=co, func=AF.Tanh, scale=0.5)
            junk = wpool.tile([P, 1], F32, tag="junk")
            nc.vector.affine_mul_reduce(out=co, accum_out=junk, in0=co,
                                        in1=cxb, scale=0.5, bias=0.5)
            # coupling addend via PE: mp = Coup^T @ th3 (PSUM)
            mp = psum.tile([P, J], F32, tag="pa")
            for i in range(J // 512):
                s0, s1 = i * 512, (i + 1) * 512
                nc.tensor.matmul(mp[:, s0:s1], w_coup, co[:, s0:s1],
                                 start=True, stop=True)
            # mu = clip(mu + mp, 0, 255)  (in place)
            nc.vector.tensor_tensor(out=mu, in0=mu, in1=mp, op=ALU.add)
            nc.vector.tensor_scalar(out=mu, in0=mu, scalar1=0.0, scalar2=255.0,
                                    op0=ALU.max, op1=ALU.min)
            # inv = exp(-sc) in place
            nc.scalar.activation(out=sc, in_=sc, func=AF.Exp, scale=-1.0)
            pre = xpool.tile([P, J], F32, tag="pre")
            nc.vector.affine_then_add(out=pre, in0=mu, in1=xb,
                                      scale=-1.0, bias=-0.5)
            # t2 = [tlo | thi]: tlo = pre*inv ; thi = tlo+inv
            t2 = wpool.tile([P, 2 * J], F32, tag="t2")
            tlo = t2[:, 0:J]
            thi = t2[:, J:2 * J]
            nc.vector.tensor_tensor(out=tlo, in0=pre, in1=sc, op=ALU.mult)
            nc.vector.tensor_tensor(out=thi, in0=tlo, in1=sc, op=ALU.add)
            # u = exp(-t); v = min(u,1e30)+1; sigma = approx_recip(v) ~2ULP
            # (single 2J-wide ops over both halves)
            nc.scalar.activation(out=t2, in_=t2, func=AF.Exp, scale=-1.0)
            nc.vector.tensor_scalar(out=t2, in0=t2, scalar1=1e30, scalar2=1.0,
                                    op0=ALU.min, op1=ALU.add)
            sfl = wpool.tile([P, 2 * J], F32, tag="sfl")
            nc.vector.reciprocal_approx_accurate(out=t2, in_=t2, scratch=sfl)
            # d = max(sig_hi - sig_lo, 1e-12) -> bf16 dm
            nc.vector.tensor_tensor(out=thi, in0=thi, in1=tlo, op=ALU.subtract)
            dm = opool.tile([P, J], BF16, tag="dm")
            nc.vector.tensor_scalar(out=dm, in0=thi, scalar1=1e-12, scalar2=None,
                                    op0=ALU.max)
            # e1 = exp(lg) -> bf16; e2 = e1*d (bf16)
            es = opool.tile([P, J], BF16, tag="es")
            nc.scalar.activation(out=es, in_=lg, func=AF.Exp)
            e2 = opool.tile([P, J], BF16, tag="e2")
            nc.vector.tensor_tensor(out=e2, in0=es, in1=dm, op=ALU.mult)

            # PE K-sums (bf16 operands) into PSUM
            rp = psum.tile([44, J], F32, tag="pb")
            for i in range(J // 512):
                s0, s1 = i * 512, (i + 1) * 512
                nc.tensor.matmul(rp[0:12, s0:s1], w_r1, es[:, s0:s1],
                                 start=True, stop=True)
                nc.tensor.matmul(rp[32:44, s0:s1], w_r2, e2[:, s0:s1],
                                 start=True, stop=True)

            # PSUM -> SBUF scratch (aligned) -> standing rbuf slice
            rsc = opool.tile([44, J], F32, tag="rsc")
            nc.scalar.copy(out=rsc, in_=rp)
            rb, tb = (rbufA, t) if t < NTA else (rbufB, t - NTA)
            nc.sync.dma_start(out=rb[tb * 12:(tb + 1) * 12, 0:J],
                              in_=rsc[0:12, :])
            nc.sync.dma_start(out=rb[tb * 12:(tb + 1) * 12, J:2 * J],
                              in_=rsc[32:44, :])
            if t == NTA - 1:
                _tail(rbufA, 0, NTA)

        _tail(rbufB, NTA, NT - NTA)

    nc.compile()
    return nc


_SELSUM = None
_SELB = None
_NC_CACHE = None


def _selsum_np():
    """fp32 coupling weights (120 cols)."""
    global _SELSUM
    if _SELSUM is None:
        m = np.zeros((P, 120), dtype=np.float32)
        cc_to_c = {0: 1, 1: 2, 2: 2}
        for cc in range(3):
            for k in range(K):
                for q in range(4):
                    p = (cc * K + k) * 4 + q
                    m[p, (cc_to_c[cc] * K + k) * 4 + q] = 1.0  # coupling
        _SELSUM = m
    return _SELSUM


def _selb_np():
    """bf16 weights: [r1 | r2 | B1 | B2] (264 cols)."""
    global _SELB
    if _SELB is None:
        import ml_dtypes
        m = np.zeros((P, 264), dtype=np.float32)
        cc_chan = {0: 0, 1: 0, 2: 1}
        for c in range(C):
            for k in range(K):
                for q in range(4):
                    p = (c * K + k) * 4 + q
                    m[p, c * 4 + q] = 1.0
                    m[p, 12 + c * 4 + q] = 1.0
                    m[c * 4 + q, 24 + p] = 1.0
                    m[cc_chan[c] * 4 + q, 144 + p] = 1.0
        _SELB = m.astype(ml_dtypes.bfloat16)
    return _SELB


def _xhl_np(x):
    """bf16 hi/lo split of x [N,C,HW] -> per-core [2*C, HW] arrays."""
    import ml_dtypes
    bf = ml_dtypes.bfloat16
    xh = x.astype(bf)
    xl = (x - xh.astype(np.float32)).astype(bf)
    return xh, xl


def _host_fixup(nll, x, l):
    """Recompute edge pixels (lo_cond/hi_cond active) exactly on host."""
    f32 = np.float32
    mask = (x < f32(0.001)) | (x > f32(254.999))
    if not mask.any():
        return nll
    l6 = l.reshape(N, 4, C, K, H, W)
    with np.errstate(over="ignore"):
        sg = lambda z: (f32(1) / (f32(1) + np.exp(-z, dtype=f32))).astype(f32)
        for n, cc, hh, ww in zip(*np.nonzero(mask)):
            s = l6[n, 0, cc, :, hh, ww]
            m_raw = l6[n, 1, :, :, hh, ww]
            sc_ = np.maximum(l6[n, 2, cc, :, hh, ww], f32(-7))
            co = sg(l6[n, 3, :, :, hh, ww])
            xpix = x[n, :, hh, ww]
            if cc == 0:
                m = m_raw[0]
            elif cc == 1:
                m = (m_raw[1] + co[0] * xpix[0]).astype(f32)
            else:
                m = (m_raw[2] + co[1] * xpix[0] + co[2] * xpix[1]).astype(f32)
            m = np.clip(m, f32(0), f32(255)).astype(f32)
            cen = (xpix[cc] - m).astype(f32)
            invv = np.exp(-sc_, dtype=f32)
            lo_c = f32(1) if xpix[cc] >= f32(0.001) else f32(0)
            hi_c = f32(1) if xpix[cc] <= f32(254.999) else f32(0)
            cdf_lo = lo_c * sg(invv * (cen - f32(0.5)))
            cdf_hi = hi_c * sg(invv * (cen + f32(0.5))) + (f32(1) - hi_c)
            d = np.maximum(cdf_hi - cdf_lo, f32(1e-12))
            e1 = np.exp(s, dtype=f32)
            e2 = (e1 * d).astype(f32)
            nll[n, cc, hh, ww] = np.log(e1.sum(dtype=f32), dtype=f32) - np.log(
                e2.sum(dtype=f32), dtype=f32)
    return nll


def _get_nc():
    global _NC_CACHE
    if _NC_CACHE is None:
        _NC_CACHE = build_kernel()
    return _NC_CACHE


def _make_in_maps(x, l):
    selsum = _selsum_np()
    selb = _selb_np()
    xh, xl = _xhl_np(x.reshape(N, C, HW))
    return [
        {"l_in": l[n].reshape(4 * CK, HW),
         "xhl_in": np.concatenate([xh[n], xl[n]], axis=0),
         "selsum_c": selsum,
         "selb_c": selb}
        for n in range(NCORES)
    ]


def kernel(x, l):
    x = np.ascontiguousarray(x, dtype=np.float32)
    l = np.ascontiguousarray(l, dtype=np.float32)
    nc = _get_nc()
    res = run_bass_kernel_spmd(nc, _make_in_maps(x, l), list(range(NCORES))).results
    nll = np.stack([res[n]["out"].reshape(C, H, W) for n in range(NCORES)], axis=0)
    return _host_fixup(nll, x, l)


# revision 9
# speedup vs baseline: 1.1436x; 1.1436x over previous
"""DiscretizedMixLogisticLoss Bass kernel for TRN2, 8-core data-parallel.

Full inputs: x [8,3,256,256] f32, l [8,120,256,256] f32 -> nll [8,3,256,256] f32.
Sharding: batch dim N=8 across 8 cores (1 example/core).

Math (per pixel, channel c, mixture k), with l viewed as [4,3,10,H*W]:
  s = l[0], mu = l[1], sc = l[2], co = l[3]
  sig3 = sigmoid(co); m' = clip(mu + coupling(sig3*x), 0, 255)
  inv = exp(-sc)
  t_lo = (x-0.5-m')*inv ; t_hi = t_lo + inv
  d = max(sigmoid(t_hi)-sigmoid(t_lo), 1e-12)
  nll = log(sum_k e^s) - log(sum_k e^s * d)
sigmoid(t) = 1/(1+exp(-t)) with the division via reciprocal_approx_accurate
(~2 ULP); the inf-guard v = min(u,1e30)+1 is fused into one tensor_scalar.
Edge pixels (x<0.001 / x>254.999, ~13 of 1.5M) are fixed up on host.

Layout per core: partition p = (c*10+k)*4 + q, where q indexes the four
16384-pixel quarters; free dim = J pixels per tile. K-sums ride TensorE via a
0/1 selection matmul (bf16 operands) into PSUM.

The loop is software-pipelined with a 2-tile skew (stage1(t) | stage2B(t-2) |
stage2A(t-1)) so the DVE never waits on the AMR->coupling-matmul->ADD chain:
by the time ADD(t-1) issues, coup(t-1) ran a full iteration earlier.
"""
from contextlib import ExitStack

import numpy as np

import concourse.bass as bass
import concourse.bacc as bacc
import concourse.tile as tile
from concourse import mybir
from concourse.bass_utils import run_bass_kernel_spmd

AF = mybir.ActivationFunctionType
ALU = mybir.AluOpType
F32 = mybir.dt.float32
BF16 = mybir.dt.bfloat16

N, C, K, H, W = 8, 3, 10, 256, 256
HW = H * W            # 65536 pixels per example
CK = C * K            # 30
P = CK * 4            # 120 partitions used
NCORES = 8

J = 1024              # pixels per partition per tile
T = 4 * J             # pixels per tile
NT = HW // T          # tiles per core (16)
QS = HW // 4          # 16384 pixel-quarter stride


def build_kernel():
    nc = bacc.Bacc("TRN2", target_bir_lowering=False, debug=False)

    l_in = nc.dram_tensor("l_in", [4 * CK, HW], F32, kind="ExternalInput")
    x_in = nc.dram_tensor("x_in", [C, HW], F32, kind="ExternalInput")
    selsum_d = nc.dram_tensor("selsum_c", [P, 384], F32, kind="ExternalInput")
    selb_d = nc.dram_tensor("selb_c", [P, 24], BF16, kind="ExternalInput")
    out = nc.dram_tensor("out", [C, HW], F32, kind="ExternalOutput")

    with tile.TileContext(nc) as tc, ExitStack() as ctx:
        consts = ctx.enter_context(tc.tile_pool(name="consts", bufs=1))
        lpool = ctx.enter_context(tc.tile_pool(name="lpool", bufs=3))
        xpool = ctx.enter_context(tc.tile_pool(name="xpool", bufs=2))
        ppool = ctx.enter_context(tc.tile_pool(name="ppool", bufs=2))
        wpool = ctx.enter_context(tc.tile_pool(name="wpool", bufs=3))
        opool = ctx.enter_context(tc.tile_pool(name="opool", bufs=3))
        psum = ctx.enter_context(tc.tile_pool(name="psum", bufs=2, space="PSUM"))
        stand = ctx.enter_context(tc.tile_pool(name="stand", bufs=1))

        selsum = consts.tile([P, 384], F32)
        nc.sync.dma_start(out=selsum, in_=selsum_d[:, :])
        selb = consts.tile([P, 24], BF16)
        nc.sync.dma_start(out=selb, in_=selb_d[:, :])
        w_coup = selsum[:, 0:120]
        w_b1 = selsum[0:12, 120:240]
        w_b2 = selsum[0:12, 240:360]
        w_r1 = selb[:, 0:12]
        w_r2 = selb[:, 12:24]

        # standing results; tails run once each group's rows are complete.
        # partition = (t - t0)*12 + (c*4+q).
        NTA = 8
        rbufA = stand.tile([NTA * 12, 2 * J], F32)
        rbufB = stand.tile([(NT - NTA) * 12, 2 * J], F32)

        def _tail(rb, t0, nt):
            nc.scalar.activation(out=rb, in_=rb, func=AF.Ln)
            nc.vector.tensor_tensor(out=rb[:, 0:J], in0=rb[:, 0:J],
                                    in1=rb[:, J:2 * J], op=ALU.subtract)
            for tt_ in range(nt):
                nc.sync.dma_start(
                    out=bass.AP(tensor=out, offset=(t0 + tt_) * J,
                                ap=[[HW, C], [QS, 4], [1, J]]),
                    in_=rb[tt_ * 12:(tt_ + 1) * 12, 0:J])

        # per-tile live state across pipeline stages
        st = {}

        def stage1(t):
            # DMAs + x broadcasts + tanh + AMR + coupling matmul
            lt = lpool.tile([P, 4 * J], F32, tag="lt")
            nc.sync.dma_start(
                out=lt,
                in_=bass.AP(tensor=l_in, offset=t * J,
                            ap=[[HW, CK], [QS, 4], [CK * HW, 4], [1, J]]))
            xq = xpool.tile([12, J], F32, tag="xq")
            nc.sync.dma_start(
                out=xq,
                in_=bass.AP(tensor=x_in, offset=t * J,
                            ap=[[HW, C], [QS, 4], [1, J]]))
            cxb = psum.tile([P, J], F32, tag="pa")
            for i in range(J // 512):
                s0, s1 = i * 512, (i + 1) * 512
                nc.tensor.matmul(cxb[:, s0:s1], w_b2, xq[:, s0:s1],
                                 start=True, stop=True)
            co = lt[:, 3 * J:4 * J]
            nc.scalar.activation(out=co, in_=co, func=AF.Tanh, scale=0.5)
            junk = wpool.tile([P, 1], F32, tag="junk")
            nc.vector.affine_mul_reduce(out=co, accum_out=junk, in0=co,
                                        in1=cxb, scale=0.5, bias=0.5)
            mp = psum.tile([P, J], F32, tag="pa")
            for i in range(J // 512):
                s0, s1 = i * 512, (i + 1) * 512
                nc.tensor.matmul(mp[:, s0:s1], w_coup, co[:, s0:s1],
                                 start=True, stop=True)
            xb = psum.tile([P, J], F32, tag="pb")
            for i in range(J // 512):
                s0, s1 = i * 512, (i + 1) * 512
                nc.tensor.matmul(xb[:, s0:s1], w_b1, xq[:, s0:s1],
                                 start=True, stop=True)
            st[t] = {"lt": lt, "mp": mp, "xb": xb}

        def stage2A(t):
            s = st[t]
            lt, mp, xb = s["lt"], s["mp"], s["xb"]
            lg = lt[:, 0:J]
            mu = lt[:, J:2 * J]
            sc = lt[:, 2 * J:3 * J]
            # mu = clip(mu + mp, 0, 255) in place
            nc.vector.tensor_tensor(out=mu, in0=mu, in1=mp, op=ALU.add)
            nc.vector.tensor_scalar(out=mu, in0=mu, scalar1=0.0, scalar2=255.0,
                                    op0=ALU.max, op1=ALU.min)
            # inv = exp(-sc) in place
            nc.scalar.activation(out=sc, in_=sc, func=AF.Exp, scale=-1.0)
            pre = ppool.tile([P, J], F32, tag="pre")
            nc.vector.affine_then_add(out=pre, in0=mu, in1=xb,
                                      scale=-1.0, bias=-0.5)
            # t2 = [tlo | thi]
            t2 = wpool.tile([P, 2 * J], F32, tag="t2")
            tlo = t2[:, 0:J]
            thi = t2[:, J:2 * J]
            nc.vector.tensor_tensor(out=tlo, in0=pre, in1=sc, op=ALU.mult)
            nc.vector.tensor_tensor(out=thi, in0=tlo, in1=sc, op=ALU.add)
            # u = exp(-t); v = min(u,1e30)+1; sigma = approx_recip(v)
            nc.scalar.activation(out=t2, in_=t2, func=AF.Exp, scale=-1.0)
            nc.vector.tensor_scalar(out=t2, in0=t2, scalar1=1e30, scalar2=1.0,
                                    op0=ALU.min, op1=ALU.add)
            sfl = wpool.tile([P, 2 * J], F32, tag="sfl")
            nc.vector.reciprocal_approx_accurate(out=t2, in_=t2, scratch=sfl)
            # es = exp(lg) -> bf16 (early: feeds e2 next stage)
            es = opool.tile([P, J], BF16, tag="es")
            nc.scalar.activation(out=es, in_=lg, func=AF.Exp)
            s["t2"] = t2
            s["es"] = es

        def stage2B(t):
            s = st.pop(t)
            t2, es = s["t2"], s["es"]
            tlo = t2[:, 0:J]
            thi = t2[:, J:2 * J]
            # d = max(sig_hi - sig_lo, 1e-12) -> bf16 dm ; e2 = es*dm
            nc.vector.tensor_tensor(out=thi, in0=thi, in1=tlo, op=ALU.subtract)
            dm = opool.tile([P, J], BF16, tag="dm")
            nc.vector.tensor_scalar(out=dm, in0=thi, scalar1=1e-12, scalar2=None,
                                    op0=ALU.max)
            e2 = opool.tile([P, J], BF16, tag="e2")
            nc.vector.tensor_tensor(out=e2, in0=es, in1=dm, op=ALU.mult)
            # PE K-sums (bf16 operands) into PSUM
            rp = psum.tile([44, J], F32, tag="pb")
            for i in range(J // 512):
                s0, s1 = i * 512, (i + 1) * 512
                nc.tensor.matmul(rp[0:12, s0:s1], w_r1, es[:, s0:s1],
                                 start=True, stop=True)
                nc.tensor.matmul(rp[32:44, s0:s1], w_r2, e2[:, s0:s1],
                                 start=True, stop=True)
            rsc = opool.tile([44, J], F32, tag="rsc")
            nc.scalar.copy(out=rsc, in_=rp)
            rb, tb = (rbufA, t) if t < NTA else (rbufB, t - NTA)
            nc.sync.dma_start(out=rb[tb * 12:(tb + 1) * 12, 0:J],
                              in_=rsc[0:12, :])
            nc.sync.dma_start(out=rb[tb * 12:(tb + 1) * 12, J:2 * J],
                              in_=rsc[32:44, :])

        for t in range(NT + 2):
            if t < NT:
                stage1(t)
            if t >= 2:
                stage2B(t - 2)
            if t >= 1 and t - 1 < NT:
                stage2A(t - 1)
            if t - 2 == NTA:  # group A rows all landed at iteration NTA+1
                _tail(rbufA, 0, NTA)

        _tail(rbufB, NTA, NT - NTA)

    nc.compile()
    return nc


_SELSUM = None
_SELB = None
_NC_CACHE = None


def _selsum_np():
    """fp32 selection/broadcast weights: [coup | B1 | B2] (384 cols)."""
    global _SELSUM
    if _SELSUM is None:
        m = np.zeros((P, 384), dtype=np.float32)
        cc_to_c = {0: 1, 1: 2, 2: 2}
        for cc in range(3):
            for k in range(K):
                for q in range(4):
                    p = (cc * K + k) * 4 + q
                    m[p, (cc_to_c[cc] * K + k) * 4 + q] = 1.0  # coupling
        # B1: xb[p=(c,k,q)] = xq[c*4+q] ; B2: cxb[p=(cc,k,q)] = xq[chan(cc)*4+q]
        cc_chan = {0: 0, 1: 0, 2: 1}
        for c in range(C):
            for k in range(K):
                for q in range(4):
                    p = (c * K + k) * 4 + q
                    m[c * 4 + q, 120 + p] = 1.0
                    m[cc_chan[c] * 4 + q, 240 + p] = 1.0
        _SELSUM = m
    return _SELSUM


def _selb_np():
    """bf16 k-sum selection weights: [r1 | r2] (24 cols)."""
    global _SELB
    if _SELB is None:
        import ml_dtypes
        m = np.zeros((P, 24), dtype=np.float32)
        for c in range(C):
            for k in range(K):
                for q in range(4):
                    p = (c * K + k) * 4 + q
                    m[p, c * 4 + q] = 1.0
                    m[p, 12 + c * 4 + q] = 1.0
        _SELB = m.astype(ml_dtypes.bfloat16)
    return _SELB


def _host_fixup(nll, x, l):
    """Recompute edge pixels (lo_cond/hi_cond active) exactly on host."""
    f32 = np.float32
    mask = (x < f32(0.001)) | (x > f32(254.999))
    if not mask.any():
        return nll
    l6 = l.reshape(N, 4, C, K, H, W)
    with np.errstate(over="ignore"):
        sg = lambda z: (f32(1) / (f32(1) + np.exp(-z, dtype=f32))).astype(f32)
        for n, cc, hh, ww in zip(*np.nonzero(mask)):
            s = l6[n, 0, cc, :, hh, ww]
            m_raw = l6[n, 1, :, :, hh, ww]
            sc_ = np.maximum(l6[n, 2, cc, :, hh, ww], f32(-7))
            co = sg(l6[n, 3, :, :, hh, ww])
            xpix = x[n, :, hh, ww]
            if cc == 0:
                m = m_raw[0]
            elif cc == 1:
                m = (m_raw[1] + co[0] * xpix[0]).astype(f32)
            else:
                m = (m_raw[2] + co[1] * xpix[0] + co[2] * xpix[1]).astype(f32)
            m = np.clip(m, f32(0), f32(255)).astype(f32)
            cen = (xpix[cc] - m).astype(f32)
            invv = np.exp(-sc_, dtype=f32)
            lo_c = f32(1) if xpix[cc] >= f32(0.001) else f32(0)
            hi_c = f32(1) if xpix[cc] <= f32(254.999) else f32(0)
            cdf_lo = lo_c * sg(invv * (cen - f32(0.5)))
            cdf_hi = hi_c * sg(invv * (cen + f32(0.5))) + (f32(1) - hi_c)
            d = np.maximum(cdf_hi - cdf_lo, f32(1e-12))
            e1 = np.exp(s, dtype=f32)
            e2 = (e1 * d).astype(f32)
            nll[n, cc, hh, ww] = np.log(e1.sum(dtype=f32), dtype=f32) - np.log(
                e2.sum(dtype=f32), dtype=f32)
    return nll


def _get_nc():
    global _NC_CACHE
    if _NC_CACHE is None:
        _NC_CACHE = build_kernel()
    return _NC_CACHE


def _make_in_maps(x, l):
    selsum = _selsum_np()
    selb = _selb_np()
    return [
        {"l_in": l[n].reshape(4 * CK, HW),
         "x_in": x[n].reshape(C, HW),
         "selsum_c": selsum,
         "selb_c": selb}
        for n in range(NCORES)
    ]


def kernel(x, l):
    x = np.ascontiguousarray(x, dtype=np.float32)
    l = np.ascontiguousarray(l, dtype=np.float32)
    nc = _get_nc()
    res = run_bass_kernel_spmd(nc, _make_in_maps(x, l), list(range(NCORES))).results
    nll = np.stack([res[n]["out"].reshape(C, H, W) for n in range(NCORES)], axis=0)
    return _host_fixup(nll, x, l)


# revision 10
# speedup vs baseline: 1.1860x; 1.0371x over previous
"""DiscretizedMixLogisticLoss Bass kernel for TRN2, 8-core data-parallel.

Full inputs: x [8,3,256,256] f32, l [8,120,256,256] f32 -> nll [8,3,256,256] f32.
Sharding: batch dim N=8 across 8 cores (1 example/core).

Math (per pixel, channel c, mixture k), with l viewed as [4,3,10,H*W]:
  s = l[0], mu = l[1], sc = l[2], co = l[3]
  sig3 = sigmoid(co); m' = clip(mu + coupling(sig3*x), 0, 255)
  inv = exp(-sc)
  t_lo = (x-0.5-m')*inv ; t_hi = t_lo + inv
  d = max(sigmoid(t_hi)-sigmoid(t_lo), 1e-12)
  nll = log(sum_k e^s) - log(sum_k e^s * d)
sigmoid(t) = 1/(1+exp(-t)) with the division via reciprocal_approx_accurate
(~2 ULP); the inf-guard v = min(u,1e30)+1 is fused into one tensor_scalar.
Edge pixels (x<0.001 / x>254.999, ~13 of 1.5M) are fixed up on host.

Layout per core: partition p = (c*10+k)*4 + q, where q indexes the four
16384-pixel quarters; free dim = J pixels per tile. K-sums ride TensorE via a
0/1 selection matmul (bf16 operands) into PSUM.

The loop is software-pipelined with a 2-tile skew (stage1(t) | stage2B(t-2) |
stage2A(t-1)) so the DVE never waits on the AMR->coupling-matmul->ADD chain:
by the time ADD(t-1) issues, coup(t-1) ran a full iteration earlier.
"""
from contextlib import ExitStack

import numpy as np

import concourse.bass as bass
import concourse.bacc as bacc
import concourse.tile as tile
from concourse import mybir
from concourse.bass_utils import run_bass_kernel_spmd

AF = mybir.ActivationFunctionType
ALU = mybir.AluOpType
F32 = mybir.dt.float32
BF16 = mybir.dt.bfloat16

N, C, K, H, W = 8, 3, 10, 256, 256
HW = H * W            # 65536 pixels per example
CK = C * K            # 30
P = CK * 4            # 120 partitions used
NCORES = 8

J = 1024              # pixels per partition per tile
T = 4 * J             # pixels per tile
NT = HW // T          # tiles per core (16)
QS = HW // 4          # 16384 pixel-quarter stride


def build_kernel():
    nc = bacc.Bacc("TRN2", target_bir_lowering=False, debug=False)

    l_in = nc.dram_tensor("l_in", [4 * CK, HW], F32, kind="ExternalInput")
    x_in = nc.dram_tensor("x_in", [C, HW], F32, kind="ExternalInput")
    selsum_d = nc.dram_tensor("selsum_c", [P, 384], F32, kind="ExternalInput")
    selb_d = nc.dram_tensor("selb_c", [P, 24], BF16, kind="ExternalInput")
    out = nc.dram_tensor("out", [C, HW], F32, kind="ExternalOutput")

    with tile.TileContext(nc) as tc, ExitStack() as ctx:
        consts = ctx.enter_context(tc.tile_pool(name="consts", bufs=1))
        lpool = ctx.enter_context(tc.tile_pool(name="lpool", bufs=3))
        xpool = ctx.enter_context(tc.tile_pool(name="xpool", bufs=2))
        ppool = ctx.enter_context(tc.tile_pool(name="ppool", bufs=2))
        wpool = ctx.enter_context(tc.tile_pool(name="wpool", bufs=3))
        opool = ctx.enter_context(tc.tile_pool(name="opool", bufs=3))
        psum = ctx.enter_context(tc.tile_pool(name="psum", bufs=2, space="PSUM"))
        stand = ctx.enter_context(tc.tile_pool(name="stand", bufs=1))

        selsum = consts.tile([P, 384], F32)
        nc.sync.dma_start(out=selsum, in_=selsum_d[:, :])
        selb = consts.tile([P, 24], BF16)
        nc.sync.dma_start(out=selb, in_=selb_d[:, :])
        w_coup = selsum[:, 0:120]
        w_b1 = selsum[0:12, 120:240]
        w_b2 = selsum[0:12, 240:360]
        w_r1 = selb[:, 0:12]
        w_r2 = selb[:, 12:24]

        # standing results; tails run once each group's rows are complete.
        # partition = (t - t0)*12 + (c*4+q).
        NTA = 8
        rbufA = stand.tile([NTA * 12, 2 * J], F32)
        rbufB = stand.tile([(NT - NTA) * 12, 2 * J], F32)

        def _tail(rb, t0, nt):
            nc.scalar.activation(out=rb, in_=rb, func=AF.Ln)
            nc.vector.tensor_tensor(out=rb[:, 0:J], in0=rb[:, 0:J],
                                    in1=rb[:, J:2 * J], op=ALU.subtract)
            for tt_ in range(nt):
                nc.sync.dma_start(
                    out=bass.AP(tensor=out, offset=(t0 + tt_) * J,
                                ap=[[HW, C], [QS, 4], [1, J]]),
                    in_=rb[tt_ * 12:(tt_ + 1) * 12, 0:J])

        # per-tile live state across pipeline stages
        st = {}

        def stage1(t):
            # DMAs + x broadcasts + tanh + AMR + coupling matmul
            lt = lpool.tile([P, 4 * J], F32, tag="lt")
            nc.sync.dma_start(
                out=lt,
                in_=bass.AP(tensor=l_in, offset=t * J,
                            ap=[[HW, CK], [QS, 4], [CK * HW, 4], [1, J]]))
            xq = xpool.tile([12, J], F32, tag="xq")
            nc.sync.dma_start(
                out=xq,
                in_=bass.AP(tensor=x_in, offset=t * J,
                            ap=[[HW, C], [QS, 4], [1, J]]))
            cxb = psum.tile([P, J], F32, tag="pa")
            for i in range(J // 512):
                s0, s1 = i * 512, (i + 1) * 512
                nc.tensor.matmul(cxb[:, s0:s1], w_b2, xq[:, s0:s1],
                                 start=True, stop=True)
            co = lt[:, 3 * J:4 * J]
            nc.scalar.activation(out=co, in_=co, func=AF.Tanh, scale=0.5)
            junk = wpool.tile([P, 1], F32, tag="junk")
            nc.vector.affine_mul_reduce(out=co, accum_out=junk, in0=co,
                                        in1=cxb, scale=0.5, bias=0.5)
            mp = psum.tile([P, J], F32, tag="pa")
            for i in range(J // 512):
                s0, s1 = i * 512, (i + 1) * 512
                nc.tensor.matmul(mp[:, s0:s1], w_coup, co[:, s0:s1],
                                 start=True, stop=True)
            xb = psum.tile([P, J], F32, tag="pb")
            for i in range(J // 512):
                s0, s1 = i * 512, (i + 1) * 512
                nc.tensor.matmul(xb[:, s0:s1], w_b1, xq[:, s0:s1],
                                 start=True, stop=True)
            st[t] = {"lt": lt, "mp": mp, "xb": xb}

        import os
        RECIP = os.environ.get("KERNEL_RECIP", "accurate")

        def stage2A1(t):
            s = st[t]
            lt, mp, xb = s["lt"], s["mp"], s["xb"]
            lg = lt[:, 0:J]
            mu = lt[:, J:2 * J]
            sc = lt[:, 2 * J:3 * J]
            # mu = clip(mu + mp, 0, 255) in place
            nc.vector.tensor_tensor(out=mu, in0=mu, in1=mp, op=ALU.add)
            nc.vector.tensor_scalar(out=mu, in0=mu, scalar1=0.0, scalar2=255.0,
                                    op0=ALU.max, op1=ALU.min)
            # inv = exp(-sc) in place
            nc.scalar.activation(out=sc, in_=sc, func=AF.Exp, scale=-1.0)
            pre = ppool.tile([P, J], F32, tag="pre")
            nc.vector.affine_then_add(out=pre, in0=mu, in1=xb,
                                      scale=-1.0, bias=-0.5)
            # t2 = [tlo | thi]
            t2 = wpool.tile([P, 2 * J], F32, tag="t2")
            tlo = t2[:, 0:J]
            thi = t2[:, J:2 * J]
            nc.vector.tensor_tensor(out=tlo, in0=pre, in1=sc, op=ALU.mult)
            nc.vector.tensor_tensor(out=thi, in0=tlo, in1=sc, op=ALU.add)
            # u = exp(-t) (2J-wide); es = exp(lg) -> bf16
            nc.scalar.activation(out=t2, in_=t2, func=AF.Exp, scale=-1.0)
            es = opool.tile([P, J], BF16, tag="es")
            nc.scalar.activation(out=es, in_=lg, func=AF.Exp)
            s["t2"] = t2
            s["es"] = es

        def stage2A2(t):
            s = st[t]
            t2 = s["t2"]
            # v = min(u,1e30)+1; sigma = approx_recip(v)
            nc.vector.tensor_scalar(out=t2, in0=t2, scalar1=1e30, scalar2=1.0,
                                    op0=ALU.min, op1=ALU.add)
            sfl = wpool.tile([P, 2 * J], F32, tag="sfl")
            if RECIP == "fast":
                nc.vector.reciprocal_approx_fast(out=sfl, in_=t2)
                s["sg"] = sfl
            else:
                nc.vector.reciprocal_approx_accurate(out=t2, in_=t2, scratch=sfl)
                s["sg"] = t2

        def stage2B(t):
            s = st.pop(t)
            sg, es = s["sg"], s["es"]
            slo = sg[:, 0:J]
            shi = sg[:, J:2 * J]
            # d = shi - slo ; e2 = max(d,1e-12)*es in one STT
            nc.vector.tensor_tensor(out=shi, in0=shi, in1=slo, op=ALU.subtract)
            e2 = opool.tile([P, J], BF16, tag="e2")
            nc.vector.scalar_tensor_tensor(out=e2, in0=shi, scalar=1e-12,
                                           in1=es, op0=ALU.max, op1=ALU.mult)
            # PE K-sums (bf16 operands) into PSUM
            rp = psum.tile([44, J], F32, tag="pb")
            for i in range(J // 512):
                s0, s1 = i * 512, (i + 1) * 512
                nc.tensor.matmul(rp[0:12, s0:s1], w_r1, es[:, s0:s1],
                                 start=True, stop=True)
                nc.tensor.matmul(rp[32:44, s0:s1], w_r2, e2[:, s0:s1],
                                 start=True, stop=True)
            rsc = opool.tile([44, J], F32, tag="rsc")
            nc.scalar.copy(out=rsc, in_=rp)
            rb, tb = (rbufA, t) if t < NTA else (rbufB, t - NTA)
            nc.sync.dma_start(out=rb[tb * 12:(tb + 1) * 12, 0:J],
                              in_=rsc[0:12, :])
            nc.sync.dma_start(out=rb[tb * 12:(tb + 1) * 12, J:2 * J],
                              in_=rsc[32:44, :])

        for t in range(NT + 2):
            if t < NT:
                stage1(t)
            if t >= 1 and t - 1 < NT:
                stage2A1(t - 1)
            if t >= 2:
                stage2B(t - 2)
            if t >= 1 and t - 1 < NT:
                stage2A2(t - 1)
            if t - 2 == NTA + 2:  # group A rows landed at iteration NTA+1
                _tail(rbufA, 0, NTA)

        _tail(rbufB, NTA, NT - NTA)

    nc.compile()
    return nc


_SELSUM = None
_SELB = None
_NC_CACHE = None


def _selsum_np():
    """fp32 selection/broadcast weights: [coup | B1 | B2] (384 cols)."""
    global _SELSUM
    if _SELSUM is None:
        m = np.zeros((P, 384), dtype=np.float32)
        cc_to_c = {0: 1, 1: 2, 2: 2}
        for cc in range(3):
            for k in range(K):
                for q in range(4):
                    p = (cc * K + k) * 4 + q
                    m[p, (cc_to_c[cc] * K + k) * 4 + q] = 1.0  # coupling
        # B1: xb[p=(c,k,q)] = xq[c*4+q] ; B2: cxb[p=(cc,k,q)] = xq[chan(cc)*4+q]
        cc_chan = {0: 0, 1: 0, 2: 1}
        for c in range(C):
            for k in range(K):
                for q in range(4):
                    p = (c * K + k) * 4 + q
                    m[c * 4 + q, 120 + p] = 1.0
                    m[cc_chan[c] * 4 + q, 240 + p] = 1.0
        _SELSUM = m
    return _SELSUM


def _selb_np():
    """bf16 k-sum selection weights: [r1 | r2] (24 cols)."""
    global _SELB
    if _SELB is None:
        import ml_dtypes
        m = np.zeros((P, 24), dtype=np.float32)
        for c in range(C):
            for k in range(K):
                for q in range(4):
                    p = (c * K + k) * 4 + q
                    m[p, c * 4 + q] = 1.0
                    m[p, 12 + c * 4 + q] = 1.0
        _SELB = m.astype(ml_dtypes.bfloat16)
    return _SELB


def _host_fixup(nll, x, l):
    """Recompute edge pixels (lo_cond/hi_cond active) exactly on host."""
    f32 = np.float32
    mask = (x < f32(0.001)) | (x > f32(254.999))
    if not mask.any():
        return nll
    l6 = l.reshape(N, 4, C, K, H, W)
    with np.errstate(over="ignore"):
        sg = lambda z: (f32(1) / (f32(1) + np.exp(-z, dtype=f32))).astype(f32)
        for n, cc, hh, ww in zip(*np.nonzero(mask)):
            s = l6[n, 0, cc, :, hh, ww]
            m_raw = l6[n, 1, :, :, hh, ww]
            sc_ = np.maximum(l6[n, 2, cc, :, hh, ww], f32(-7))
            co = sg(l6[n, 3, :, :, hh, ww])
            xpix = x[n, :, hh, ww]
            if cc == 0:
                m = m_raw[0]
            elif cc == 1:
                m = (m_raw[1] + co[0] * xpix[0]).astype(f32)
            else:
                m = (m_raw[2] + co[1] * xpix[0] + co[2] * xpix[1]).astype(f32)
            m = np.clip(m, f32(0), f32(255)).astype(f32)
            cen = (xpix[cc] - m).astype(f32)
            invv = np.exp(-sc_, dtype=f32)
            lo_c = f32(1) if xpix[cc] >= f32(0.001) else f32(0)
            hi_c = f32(1) if xpix[cc] <= f32(254.999) else f32(0)
            cdf_lo = lo_c * sg(invv * (cen - f32(0.5)))
            cdf_hi = hi_c * sg(invv * (cen + f32(0.5))) + (f32(1) - hi_c)
            d = np.maximum(cdf_hi - cdf_lo, f32(1e-12))
            e1 = np.exp(s, dtype=f32)
            e2 = (e1 * d).astype(f32)
            nll[n, cc, hh, ww] = np.log(e1.sum(dtype=f32), dtype=f32) - np.log(
                e2.sum(dtype=f32), dtype=f32)
    return nll


def _get_nc():
    global _NC_CACHE
    if _NC_CACHE is None:
        _NC_CACHE = build_kernel()
    return _NC_CACHE


def _make_in_maps(x, l):
    selsum = _selsum_np()
    selb = _selb_np()
    return [
        {"l_in": l[n].reshape(4 * CK, HW),
         "x_in": x[n].reshape(C, HW),
         "selsum_c": selsum,
         "selb_c": selb}
        for n in range(NCORES)
    ]


def kernel(x, l):
    x = np.ascontiguousarray(x, dtype=np.float32)
    l = np.ascontiguousarray(l, dtype=np.float32)
    nc = _get_nc()
    res = run_bass_kernel_spmd(nc, _make_in_maps(x, l), list(range(NCORES))).results
    nll = np.stack([res[n]["out"].reshape(C, H, W) for n in range(NCORES)], axis=0)
    return _host_fixup(nll, x, l)
